# revision 3
# baseline (speedup 1.0000x reference)
"""GCF message passing on 8 trn2 cores — V2.

Tables are bf16 [N,128] DRAM; table_0 comes from the host. Per layer l:
  SpMM  - per core: edges bucketed by cell=(128-row dest block, col range).
          One dma_gather per (superblock of 1024 dest rows, range) covering
          that call's chunks (128 edge slots each; padded slots have
          val=0 so they contribute nothing). Per chunk: one-hot
          O[e, 0:128] (bf16), matmul psum[128d, 1024] += Xg^T @ O at the
          chunk's structural 128-wide window. PSUM superblock is opened by
          a ones[1,1]x zeros[1,1024] matmul and drained once per sb to
          SBUF Lx^T (bf16) by the scalar engine.
  Dense - transposed layout: y^T = Wlin^T@(Lx+F)^T + Wint^T@(Lx*F)^T,
          fused bias+lrelu on ACT, row-norm via ones-matmul, reciprocal,
          K=1 broadcast matmul; PE transposes -> bf16 rows -> DRAM fshard.
  Share - AllGather bf16 fshard -> table_{l+1}.
  Final - level l+1 partial dots right after the share: gather u/i rows
          from table_{l+1} (bucketed by range), multiply+reduce, accumulate.
          Level 0 is computed on the host and added after the run.
"""

import os

import numpy as np
import ml_dtypes

import concourse.bacc as bacc
import concourse.mybir as mybir
import concourse.tile as tile
from concourse.bass import ts
from concourse.bass_utils import run_bass_kernel_spmd
from concourse.masks import make_identity

NUM_USERS = 30000
NUM_ITEMS = 70000
N = 100000
D = 128
NL = 3
BATCH = 16384
NCORE = 8
SHARD = N // NCORE            # 12500
SB = 512                      # dest rows per PSUM bank tile
NSB = (SHARD + SB - 1) // SB  # 25
GSB = 1024                    # dest rows per gather call
NGSB = (SHARD + GSB - 1) // GSB  # 13
BLK = 128                     # dest rows per one-hot window
NBLK = (SHARD + BLK - 1) // BLK  # 98
RB = 512                      # dense-phase block
NB = (SHARD + RB - 1) // RB   # 25
RS = 32768                    # gather index range (int16)
NR = 4
BSH = BATCH // NCORE          # 2048
EPS = 1e-12
SLOPE = 0.01

f32 = mybir.dt.float32
f32r = mybir.dt.float32r
bf16 = mybir.dt.bfloat16
i16 = mybir.dt.int16

_cache = {}

STAGE = int(os.environ.get("K2STAGE", "5"))
NLAYERS = int(os.environ.get("K2NL", str(NL)))


def _build(meta):
    (calls, chunks, NIC, NCH, NFB, fin_bounds) = meta
    # calls: (rng, sb, idxcol0, n_chunks)   -- one dma_gather each
    # chunks: (call_i, sub_i, w, sb_open, sb_close, sb_i)
    nc = bacc.Bacc(num_devices=NCORE)

    tab0 = nc.dram_tensor("tab0", [N, D], bf16, kind="ExternalInput")
    f0t = nc.dram_tensor("f0t", [D, SHARD], bf16, kind="ExternalInput")
    eidx = nc.dram_tensor("eidx", [128, NIC], i16, kind="ExternalInput")
    erow = nc.dram_tensor("erow", [128, NCH], f32, kind="ExternalInput")
    evals = nc.dram_tensor("evals", [128, NCH], f32, kind="ExternalInput")
    wlin = nc.dram_tensor("wlin", [D, NL * D], f32, kind="ExternalInput")
    wint = nc.dram_tensor("wint", [D, NL * D], f32, kind="ExternalInput")
    biasc = nc.dram_tensor("biasc", [D, NL], f32, kind="ExternalInput")
    uidx = nc.dram_tensor("uidx", [128, NFB * 8], i16, kind="ExternalInput")
    iidx = nc.dram_tensor("iidx", [128, NFB * 8], i16, kind="ExternalInput")
    score = nc.dram_tensor("score", [128, NFB], f32, kind="ExternalOutput")
    fsh_out = nc.dram_tensor("fsh_out", [SHARD, D], bf16,
                             kind="ExternalOutput")

    add = mybir.AluOpType.add
    mult = mybir.AluOpType.mult
    is_equal = mybir.AluOpType.is_equal
    maxop = mybir.AluOpType.max
    AF = mybir.ActivationFunctionType

    with tile.TileContext(nc) as tc:
        with (
            tc.tile_pool(name="const", bufs=1) as cp,
            tc.tile_pool(name="ft", bufs=1) as ftp,
            tc.tile_pool(name="g", bufs=5) as gp,
            tc.tile_pool(name="fin", bufs=1) as fp_,
            tc.tile_pool(name="ot", bufs=6) as otp,
            tc.tile_pool(name="sb", bufs=3) as sbp,
            tc.tile_pool(name="psb", bufs=2, space="PSUM") as psb,
            tc.tile_pool(name="py", bufs=2, space="PSUM") as pyp,
            tc.tile_pool(name="pn", bufs=1, space="PSUM") as pnp,
            tc.tile_pool(name="pbc", bufs=1, space="PSUM") as pbc,
            tc.tile_pool(name="ptp", bufs=2, space="PSUM") as ptp,
            tc.tile_pool(name="dram", bufs=1, space="DRAM") as dp,
        ):
            # ---------- constants ----------
            iota_b = cp.tile([128, 2 * BLK], bf16)
            nc.gpsimd.iota(iota_b[:], pattern=[[1, 2 * BLK]], base=0,
                           channel_multiplier=0,
                           allow_small_or_imprecise_dtypes=True)
            ident = cp.tile([128, 128], bf16)
            make_identity(nc, ident[:])
            ones_f = cp.tile([128, 1], f32)
            nc.vector.memset(ones_f[:], 1.0)
            ones_b = cp.tile([128, 1], bf16)
            nc.vector.tensor_copy(ones_b[:], ones_f[:])
            onesrow_f = cp.tile([1, 128], f32)
            nc.vector.memset(onesrow_f[:], 1.0)
            onesrow_r = cp.tile([1, 128], f32r)
            nc.vector.tensor_copy(onesrow_r[:], onesrow_f[:])
            ones1_f = cp.tile([1, 1], f32)
            nc.vector.memset(ones1_f[:], 1.0)
            ones1_r = cp.tile([1, 1], f32r)
            nc.vector.tensor_copy(ones1_r[:], ones1_f[:])
            zrow_f = cp.tile([1, SB], f32)
            nc.vector.memset(zrow_f[:], 0.0)
            zrow_b = cp.tile([1, SB], f32r)
            nc.vector.tensor_copy(zrow_b[:], zrow_f[:])

            wlin_f = cp.tile([128, NL * 128], f32)
            nc.sync.dma_start(out=wlin_f[:], in_=wlin[:])
            wlin_r = cp.tile([128, NL * 128], bf16)
            nc.vector.tensor_copy(wlin_r[:], wlin_f[:])
            wint_f = cp.tile([128, NL * 128], f32)
            nc.sync.dma_start(out=wint_f[:], in_=wint[:])
            wint_r = cp.tile([128, NL * 128], bf16)
            nc.vector.tensor_copy(wint_r[:], wint_f[:])
            bias_sb = cp.tile([128, NL], f32)
            nc.sync.dma_start(out=bias_sb[:], in_=biasc[:])

            eidx_sb = cp.tile([128, NIC], i16)
            nc.sync.dma_start(out=eidx_sb[:], in_=eidx[:])
            erow_sb = cp.tile([128, NCH], f32)
            nc.sync.dma_start(out=erow_sb[:], in_=erow[:])
            evals_sb = cp.tile([128, NCH], f32)
            nc.sync.dma_start(out=evals_sb[:], in_=evals[:])
            uidx_sb = cp.tile([128, NFB * 8], i16)
            nc.sync.dma_start(out=uidx_sb[:], in_=uidx[:])
            iidx_sb = cp.tile([128, NFB * 8], i16)
            nc.sync.dma_start(out=iidx_sb[:], in_=iidx[:])

            fta = ftp.tile([128, NB * RB], bf16, tag="fta")
            ftb = ftp.tile([128, NB * RB], bf16, tag="ftb")
            nc.vector.memset(fta[:, SHARD:], 0.0)
            nc.sync.dma_start(out=fta[:, :SHARD], in_=f0t[:])
            lxt = ftp.tile([128, NB * RB], bf16, tag="lxt")
            nc.vector.memset(lxt[:, SHARD:], 0.0)

            fshard = dp.tile([SHARD, D], bf16)
            tabs = [
                dp.tile([N, D], bf16, name=f"tab{1 + i}", tag=f"tab{1 + i}",
                        addr_space="Shared")
                for i in range(NL)
            ]

            acc = cp.tile([128, NFB], f32)
            nc.vector.memset(acc[:], 0.0)

            n_layers = NLAYERS if STAGE >= 1 else 0
            for l in range(n_layers):
                ftin = fta if l % 2 == 0 else ftb
                ftout = ftb if l % 2 == 0 else fta
                src = tab0 if l == 0 else tabs[l - 1]

                # ---------- SpMM ----------
                gts = []
                for (rng, sb_i, c0, nq) in calls:
                    gt = gp.tile([128, nq * 128], bf16, tag="g",
                                 name=f"g{l}_{c0}")
                    nc.gpsimd.dma_gather(
                        gt[:].rearrange("p (c d) -> p c d", d=128),
                        src[rng * RS:, :],
                        eidx_sb[:, c0:c0 + nq * 8],
                        nq * 128, nq * 128, 128,
                        single_packet=False,
                    )
                    gts.append(gt)
                cur = None
                for qi, (call_i, sub_i, w, sb_open, sb_close, sb_i,
                         wide) in enumerate(chunks):
                    gt = gts[call_i]
                    if sb_open:
                        cur = psb.tile([128, SB], f32, tag="psb",
                                       name=f"psb{l}_{sb_i}")
                        nc.tensor.matmul(out=cur[:], lhsT=onesrow_r[:],
                                         rhs=zrow_b[:], start=True,
                                         stop=False)
                    ow = 2 * BLK if wide else BLK
                    o = otp.tile([128, 2 * BLK], bf16, tag="o",
                                 name=f"o{l}_{qi}")
                    nc.vector.tensor_scalar(
                        out=o[:, :ow], in0=iota_b[:, :ow],
                        scalar1=erow_sb[:, qi:qi + 1],
                        scalar2=evals_sb[:, qi:qi + 1],
                        op0=is_equal, op1=mult,
                    )
                    nc.tensor.matmul(
                        out=cur[:, w:w + ow],
                        lhsT=gt[:, ts(sub_i, 128)],
                        rhs=o[:, :ow],
                        start=False, stop=bool(sb_close),
                    )
                    if sb_close:
                        nsb = min(SB, SHARD - sb_i * SB)
                        nc.scalar.activation(
                            out=lxt[:, sb_i * SB:sb_i * SB + nsb],
                            in_=cur[:, :nsb], func=AF.Copy)

                # ---------- dense ----------
                for b in range(NB if STAGE >= 2 else 0):
                    lxs = lxt[:, ts(b, RB)]
                    fin_ = ftin[:, ts(b, RB)]
                    pre1 = sbp.tile([128, RB], bf16, tag="pre1")
                    nc.vector.tensor_tensor(out=pre1[:], in0=lxs, in1=fin_,
                                            op=add)
                    pre2 = sbp.tile([128, RB], bf16, tag="pre2")
                    nc.vector.tensor_tensor(out=pre2[:], in0=lxs, in1=fin_,
                                            op=mult)
                    y = pyp.tile([128, RB], f32, tag="y")
                    nc.tensor.matmul(out=y[:], lhsT=wlin_r[:, ts(l, 128)],
                                     rhs=pre1[:], start=True, stop=False)
                    nc.tensor.matmul(out=y[:], lhsT=wint_r[:, ts(l, 128)],
                                     rhs=pre2[:], start=False, stop=True)
                    ya = sbp.tile([128, RB], bf16, tag="ya")
                    nc.scalar.activation(out=ya[:], in_=y[:], func=AF.Lrelu,
                                         bias=bias_sb[:, l:l + 1], scale=1.0,
                                         alpha=SLOPE)
                    sq = sbp.tile([128, RB], bf16, tag="sq")
                    nc.vector.tensor_tensor(out=sq[:], in0=ya[:], in1=ya[:],
                                            op=mult)
                    nsq = pnp.tile([1, RB], f32, tag="nsq")
                    nc.tensor.matmul(out=nsq[:], lhsT=ones_b[:], rhs=sq[:],
                                     start=True, stop=True)
                    rt = sbp.tile([1, RB], f32, tag="rt")
                    nc.scalar.activation(out=rt[:], in_=nsq[:], func=AF.Sqrt)
                    nc.vector.tensor_scalar(out=rt[:], in0=rt[:],
                                            scalar1=EPS, scalar2=None,
                                            op0=maxop)
                    inv = sbp.tile([1, RB], f32r, tag="inv")
                    with nc.allow_low_precision(reason="f32r broadcast input"):
                        nc.vector.reciprocal(inv[:], rt[:])
                    bc = pbc.tile([128, RB], f32, tag="bc")
                    nc.tensor.matmul(out=bc[:], lhsT=onesrow_r[:], rhs=inv[:],
                                     start=True, stop=True)
                    nc.vector.tensor_tensor(out=ftout[:, ts(b, RB)],
                                            in0=ya[:], in1=bc[:], op=mult)
                    for h in range(4):
                        r0 = b * RB + h * 128
                        nrr = min(128, SHARD - r0)
                        if nrr <= 0:
                            break
                        tp = ptp.tile([128, 128], bf16, tag="tp",
                                      name=f"tp{l}_{b}_{h}")
                        nc.tensor.transpose(
                            out=tp[:], in_=ftout[:, r0:r0 + 128],
                            identity=ident[:])
                        cpo = sbp.tile([128, 128], bf16, tag="cpo")
                        nc.scalar.activation(out=cpo[:], in_=tp[:],
                                             func=AF.Copy)
                        nc.sync.dma_start(
                            out=(fshard if l < NL - 1 else fsh_out)
                            [r0:r0 + nrr, :],
                            in_=cpo[:nrr, :])

                if STAGE >= 3 and l < NL - 1:
                    nc.gpsimd.collective_compute(
                        "AllGather", mybir.AluOpType.bypass,
                        replica_groups=[list(range(NCORE))],
                        ins=[fshard.opt()], outs=[tabs[l].opt()],
                    )

                # ---------- final: level l+1 partial dots ----------
                if STAGE >= 5 and l < NL - 1:
                    srcf = tabs[l]
                    ug = fp_.tile([128, NFB * 128], bf16, tag="ug",
                                  name=f"ug{l}")
                    nc.gpsimd.dma_gather(
                        ug[:].rearrange("p (c d) -> p c d", d=128),
                        srcf[:],
                        uidx_sb[:],
                        NFB * 128, NFB * 128, 128,
                        single_packet=False,
                    )
                    ig = fp_.tile([128, NFB * 128], bf16, tag="ig",
                                  name=f"ig{l}")
                    for (rr, cc0, cn) in fin_bounds:
                        nc.gpsimd.dma_gather(
                            ig[:, cc0 * 128:(cc0 + cn) * 128].rearrange(
                                "p (c d) -> p c d", d=128),
                            srcf[rr * RS:, :],
                            iidx_sb[:, cc0 * 8:(cc0 + cn) * 8],
                            cn * 128, cn * 128, 128,
                            single_packet=False,
                        )
                    nc.vector.tensor_tensor(out=ug[:], in0=ug[:],
                                            in1=ig[:], op=mult)
                    sc = sbp.tile([128, NFB], f32, tag="sc")
                    nc.vector.tensor_reduce(
                        out=sc[:],
                        in_=ug[:].rearrange("p (c d) -> p c d", d=128),
                        axis=mybir.AxisListType.X, op=add)
                    nc.vector.tensor_tensor(out=acc[:], in0=acc[:],
                                            in1=sc[:], op=add)
            nc.sync.dma_start(out=score[:], in_=acc[:])

    nc.compile()
    return nc


def _pack_inputs(userIdx, itemIdx, rows, cols, vals, uEmbd, iEmbd,
                 Wlin, blin, Wint, bint):
    rows = np.asarray(rows, dtype=np.int64)
    cols = np.asarray(cols, dtype=np.int64)
    vals = np.asarray(vals, dtype=np.float32)
    userIdx = np.asarray(userIdx, dtype=np.int64)
    itemIdx = np.asarray(itemIdx, dtype=np.int64)

    feat0 = np.ascontiguousarray(
        np.concatenate([np.asarray(uEmbd, np.float32),
                        np.asarray(iEmbd, np.float32)], axis=0))
    tab0 = np.ascontiguousarray(feat0.astype(ml_dtypes.bfloat16))

    # ---- edge bucketing: cell = (core, blk, rng) ----
    core = rows // SHARD
    local = rows - core * SHARD
    blk = local // BLK
    rowl = (local - blk * BLK).astype(np.float32)
    rng = cols // RS
    col_local = (cols - rng * RS).astype(np.int16)

    ckey = ((core * NBLK + blk) * NR + rng).astype(np.int64)
    order = np.argsort(ckey, kind="stable")
    ckey_s = ckey[order]
    counts = np.bincount(ckey_s, minlength=NCORE * NBLK * NR)
    counts = counts.reshape(NCORE, NBLK, NR)

    # structure: gather call = (gsb, rng); psum group = sb of 512 rows.
    # Cells (blk128, rng) get floor(max8/128) full 128-wide chunks; the
    # leftovers of each aligned block PAIR (2b, 2b+1) share merged chunks
    # with a 256-wide one-hot window (256-aligned, so never straddling the
    # 512-col psum tile). Slot order within a call and chunk emission
    # order are identical: for sb -> for pair -> fulls(2b), fulls(2b+1),
    # merged.
    cnt_max = counts.max(axis=0)  # [NBLK, NR]
    F = cnt_max // 128            # full chunks per cell
    L = cnt_max - F * 128         # leftover per cell (0..127)
    NPAIR = NBLK // 2
    M = np.zeros((NPAIR, NR), dtype=np.int64)      # merged wide chunks
    SGL = np.zeros((NBLK, NR), dtype=np.int64)     # solo narrow leftover
    for p in range(NPAIR):
        a, b = 2 * p, 2 * p + 1
        do_merge = (L[a] > 0) & (L[b] > 0) & (L[a] + L[b] <= 128)
        M[p] = np.where(do_merge, 1, 0)
        SGL[a] = np.where(do_merge, 0, (L[a] > 0).astype(np.int64))
        SGL[b] = np.where(do_merge, 0, (L[b] > 0).astype(np.int64))

    FT = F + SGL        # narrow chunks per cell (fulls + solo leftover)
    calls = []          # (rng, gsb, idxcol0, n_slots)
    chunks = []         # (call_i, sub_i, w, sb_open, sb_close, sb_i, wide)
    full_sub = np.zeros((NBLK, NR), dtype=np.int64)   # sub_i of cell fulls
    merge_sub = np.zeros((NPAIR, NR), dtype=np.int64)  # sub_i of merged
    call_of_rng = {}
    idxcol = 0
    for g in range(NGSB):
        blk_g_lo = g * (GSB // BLK)
        blk_g_hi = min(NBLK, blk_g_lo + GSB // BLK)
        for r in range(NR):
            call_nq = 0
            for p in range(blk_g_lo // 2, (blk_g_hi + 1) // 2):
                for b in (2 * p, 2 * p + 1):
                    if b >= NBLK:
                        continue
                    full_sub[b, r] = call_nq
                    call_nq += int(FT[b, r])
                merge_sub[p, r] = call_nq
                call_nq += int(M[p, r])
            if call_nq == 0:
                continue
            call_of_rng[(g, r)] = (len(calls), idxcol)
            calls.append((r, g, idxcol, call_nq))
            idxcol += call_nq * 8
        # chunk emission: sb-major
        sb_lo = g * (GSB // SB)
        sb_hi = min(NSB, sb_lo + GSB // SB)
        for sbi in range(sb_lo, sb_hi):
            blk_lo = sbi * (SB // BLK)
            blk_hi = min(NBLK, blk_lo + SB // BLK)
            sb_chunks = []
            for r in range(NR):
                if (g, r) not in call_of_rng:
                    continue
                call_i, _ = call_of_rng[(g, r)]
                for p in range(blk_lo // 2, (blk_hi + 1) // 2):
                    for b in (2 * p, 2 * p + 1):
                        if b >= NBLK:
                            continue
                        for j in range(int(FT[b, r])):
                            sb_chunks.append(
                                (call_i, int(full_sub[b, r]) + j,
                                 (b - blk_lo) * BLK, False))
                    for j in range(int(M[p, r])):
                        sb_chunks.append(
                            (call_i, int(merge_sub[p, r]) + j,
                             (2 * p - blk_lo) * BLK, True))
            for t, (call_i, sub_i, w, wide) in enumerate(sb_chunks):
                chunks.append((call_i, sub_i, w, t == 0,
                               t == len(sb_chunks) - 1, sbi, wide))
    NCH = len(chunks)
    NIC = idxcol

    # ---- scatter edges into slots ----
    # per-cell chunk id (emission order) + idx col base for fulls; per-pair
    # for merged. A core's first 128*F[b,r] edges of cell (b,r) fill the
    # full chunks; remaining edges go to the pair's merged chunks after the
    # sibling cell's leftovers offset (per-core running offset).
    full_chunk = np.zeros((NBLK, NR), dtype=np.int64)
    full_icol = np.zeros((NBLK, NR), dtype=np.int64)
    merge_chunk = np.zeros((NPAIR, NR), dtype=np.int64)
    merge_icol = np.zeros((NPAIR, NR), dtype=np.int64)
    qi = 0
    for g in range(NGSB):
        sb_lo = g * (GSB // SB)
        sb_hi = min(NSB, sb_lo + GSB // SB)
        for sbi in range(sb_lo, sb_hi):
            blk_lo = sbi * (SB // BLK)
            blk_hi = min(NBLK, blk_lo + SB // BLK)
            for r in range(NR):
                if (g, r) not in call_of_rng:
                    continue
                call_i, ic0 = call_of_rng[(g, r)]
                for p in range(blk_lo // 2, (blk_hi + 1) // 2):
                    for b in (2 * p, 2 * p + 1):
                        if b >= NBLK:
                            continue
                        if FT[b, r]:
                            full_chunk[b, r] = qi
                            full_icol[b, r] = ic0 + int(full_sub[b, r]) * 8
                        qi += int(FT[b, r])
                    if M[p, r]:
                        merge_chunk[p, r] = qi
                        merge_icol[p, r] = ic0 + int(merge_sub[p, r]) * 8
                    qi += int(M[p, r])
    assert qi == NCH, (qi, NCH)

    starts = np.zeros(NCORE * NBLK * NR, dtype=np.int64)
    np.cumsum(counts.reshape(-1)[:-1], out=starts[1:])
    pos = np.arange(len(ckey_s), dtype=np.int64) - starts[ckey_s]
    core_s = core[order]
    blk_s = blk[order]
    rng_s = rng[order]
    pair_s = blk_s // 2
    fcap = (FT * 128)[blk_s, rng_s]         # narrow-chunk capacity of cell
    is_full = pos < fcap
    # merged position: sibling-leftover offset for odd blocks (per core)
    sib = blk_s ^ 1
    sib_left = np.where(
        (blk_s % 2 == 1) & (sib < NBLK),
        np.maximum(counts[core_s, sib, rng_s] - (FT * 128)[sib, rng_s], 0),
        0)
    mpos = np.where(is_full, 0, pos - fcap + sib_left)
    q_of_edge = np.where(
        is_full,
        full_chunk[blk_s, rng_s] + pos // 128,
        merge_chunk[pair_s, rng_s] + mpos // 128)
    icol_of_edge = np.where(
        is_full,
        full_icol[blk_s, rng_s] + (pos // 16) % 8 + (pos // 128) * 8,
        merge_icol[pair_s, rng_s] + (mpos // 16) % 8 + (mpos // 128) * 8)
    p_of_edge = np.where(is_full, pos % 128, mpos % 128)
    # erow: window-relative dest. fulls: blk-local [0,128); merged: relative
    # to the pair's even block [0,256)
    roww = np.where(is_full, rowl[order],
                    rowl[order] + (blk_s % 2) * 128)

    eidx_arr = np.zeros((NCORE, 16, NIC), dtype=np.int16)
    erow_arr = np.zeros((NCORE, 128, NCH), dtype=np.float32)
    eval_arr = np.zeros((NCORE, 128, NCH), dtype=np.float32)
    eidx_arr[core_s, p_of_edge % 16, icol_of_edge] = col_local[order]
    erow_arr[core_s, p_of_edge, q_of_edge] = roww
    eval_arr[core_s, p_of_edge, q_of_edge] = vals[order]

    # ---- weights ----
    wlin_h = np.ascontiguousarray(
        np.asarray(Wlin, np.float32).transpose(1, 0, 2).reshape(D, NL * D))
    wint_h = np.ascontiguousarray(
        np.asarray(Wint, np.float32).transpose(1, 0, 2).reshape(D, NL * D))
    biasc = np.ascontiguousarray(
        (np.asarray(blin, np.float32) + np.asarray(bint, np.float32)).T)

    # ---- final stage: bucket item rows by range ----
    irow = itemIdx + NUM_USERS
    ir = irow // RS
    nfb_counts = np.zeros((NCORE, NR), dtype=np.int64)
    perms = []
    for c in range(NCORE):
        sl = slice(c * BSH, (c + 1) * BSH)
        o = np.argsort(ir[sl], kind="stable")
        perms.append(o)
        nfb_counts[c] = np.bincount(ir[sl][o], minlength=NR)
    bucket_chunks = np.ceil(nfb_counts.max(axis=0) / 128).astype(np.int64)
    fin_bounds = []
    c0 = 0
    for r in range(NR):
        n = int(bucket_chunks[r])
        if n == 0:
            continue
        fin_bounds.append((r, c0, n))
        c0 += n
    NFB = c0

    uidx_arr = np.zeros((NCORE, 16, NFB * 8), dtype=np.int16)
    iidx_arr = np.zeros((NCORE, 16, NFB * 8), dtype=np.int16)
    inv_perm = np.full((NCORE, NFB * 128), -1, dtype=np.int64)
    for c in range(NCORE):
        sl = slice(c * BSH, (c + 1) * BSH)
        o = perms[c]
        u_s = userIdx[sl][o]
        i_s = irow[sl][o]
        r_s = ir[sl][o]
        jpos = np.zeros(BSH, dtype=np.int64)
        for (r, b0, nchk) in fin_bounds:
            m = r_s == r
            jpos[m] = b0 * 128 + np.arange(int(m.sum()))
        uidx_arr[c, jpos % 16, (jpos // 128) * 8 + (jpos % 128) // 16] = \
            u_s.astype(np.int16)
        iidx_arr[c, jpos % 16, (jpos // 128) * 8 + (jpos % 128) // 16] = \
            (i_s - r_s * RS).astype(np.int16)
        inv_perm[c, jpos] = np.arange(c * BSH, (c + 1) * BSH)[o]

    # host-side level-0 contribution (exact f32); level-3 added after run
    score0 = np.sum(feat0[userIdx] * feat0[irow], axis=1).astype(np.float32)

    meta = (tuple(calls), tuple(chunks), NIC, NCH, NFB, tuple(fin_bounds))

    in_maps = []
    for c in range(NCORE):
        f0t = np.ascontiguousarray(
            feat0[c * SHARD:(c + 1) * SHARD].T.astype(ml_dtypes.bfloat16))
        in_maps.append({
            "tab0": tab0,
            "f0t": f0t,
            "eidx": np.ascontiguousarray(np.tile(eidx_arr[c], (8, 1))),
            "erow": np.ascontiguousarray(erow_arr[c]),
            "evals": np.ascontiguousarray(eval_arr[c]),
            "wlin": wlin_h,
            "wint": wint_h,
            "biasc": biasc,
            "uidx": np.ascontiguousarray(np.tile(uidx_arr[c], (8, 1))),
            "iidx": np.ascontiguousarray(np.tile(iidx_arr[c], (8, 1))),
        })
    return meta, in_maps, inv_perm, score0


def kernel(**inputs) -> np.ndarray:
    meta, in_maps, inv_perm, score0 = _pack_inputs(**inputs)
    key = meta[:4] + (meta[4], meta[5])
    if key not in _cache:
        _cache[key] = _build(meta)
    nc = _cache[key]
    res = run_bass_kernel_spmd(nc, in_maps, list(range(NCORE)))
    out = np.empty(BATCH, dtype=np.float32)
    NFB = meta[4]
    for c in range(NCORE):
        sc = res.results[c]["score"]  # [128, NFB]
        vals_j = sc[np.arange(NFB * 128) % 128, np.arange(NFB * 128) // 128]
        valid = inv_perm[c] >= 0
        out[inv_perm[c][valid]] = vals_j[valid]
    f3 = np.concatenate(
        [np.asarray(res.results[c]["fsh_out"]).astype(np.float32)
         for c in range(NCORE)], axis=0)
    userIdx = np.asarray(inputs["userIdx"], dtype=np.int64)
    itemIdx = np.asarray(inputs["itemIdx"], dtype=np.int64)
    score3 = np.sum(f3[userIdx] * f3[itemIdx + NUM_USERS], axis=1)
    return out + score0 + score3.astype(np.float32)


# revision 4
# speedup vs baseline: 1.0017x; 1.0017x over previous
"""GCF message passing on 8 trn2 cores — V2.

Tables are bf16 [N,128] DRAM; table_0 comes from the host. Per layer l:
  SpMM  - per core: edges bucketed by cell=(128-row dest block, col range).
          One dma_gather per (superblock of 1024 dest rows, range) covering
          that call's chunks (128 edge slots each; padded slots have
          val=0 so they contribute nothing). Per chunk: one-hot
          O[e, 0:128] (bf16), matmul psum[128d, 1024] += Xg^T @ O at the
          chunk's structural 128-wide window. PSUM superblock is opened by
          a ones[1,1]x zeros[1,1024] matmul and drained once per sb to
          SBUF Lx^T (bf16) by the scalar engine.
  Dense - transposed layout: y^T = Wlin^T@(Lx+F)^T + Wint^T@(Lx*F)^T,
          fused bias+lrelu on ACT, row-norm via ones-matmul, reciprocal,
          K=1 broadcast matmul; PE transposes -> bf16 rows -> DRAM fshard.
  Share - AllGather bf16 fshard -> table_{l+1}.
  Final - level l+1 partial dots right after the share: gather u/i rows
          from table_{l+1} (bucketed by range), multiply+reduce, accumulate.
          Level 0 is computed on the host and added after the run.
"""

import os

import numpy as np
import ml_dtypes

import concourse.bacc as bacc
import concourse.mybir as mybir
import concourse.tile as tile
from concourse.bass import ts
from concourse.bass_utils import run_bass_kernel_spmd
from concourse.masks import make_identity

NUM_USERS = 30000
NUM_ITEMS = 70000
N = 100000
D = 128
NL = 3
BATCH = 16384
NCORE = 8
SHARD = N // NCORE            # 12500
SB = 512                      # dest rows per PSUM bank tile
NSB = (SHARD + SB - 1) // SB  # 25
GSB = 1024                    # dest rows per gather call
NGSB = (SHARD + GSB - 1) // GSB  # 13
BLK = 128                     # dest rows per one-hot window
NBLK = (SHARD + BLK - 1) // BLK  # 98
RB = 512                      # dense-phase block
NB = (SHARD + RB - 1) // RB   # 25
RS = 32768                    # gather index range (int16)
NR = 4
BSH = BATCH // NCORE          # 2048
EPS = 1e-12
SLOPE = 0.01

f32 = mybir.dt.float32
f32r = mybir.dt.float32r
bf16 = mybir.dt.bfloat16
i16 = mybir.dt.int16

_cache = {}

STAGE = int(os.environ.get("K2STAGE", "5"))
NLAYERS = int(os.environ.get("K2NL", str(NL)))


def _build(meta):
    (calls, chunks, NIC, NCH, NFB, fin_bounds) = meta
    # calls: (rng, sb, idxcol0, n_chunks)   -- one dma_gather each
    # chunks: (call_i, sub_i, w, sb_open, sb_close, sb_i)
    nc = bacc.Bacc(num_devices=NCORE)

    tab0 = nc.dram_tensor("tab0", [N, D], bf16, kind="ExternalInput")
    f0t = nc.dram_tensor("f0t", [D, SHARD], bf16, kind="ExternalInput")
    eidx = nc.dram_tensor("eidx", [128, NIC], i16, kind="ExternalInput")
    erow = nc.dram_tensor("erow", [128, NCH], f32, kind="ExternalInput")
    evals = nc.dram_tensor("evals", [128, NCH], f32, kind="ExternalInput")
    wlin = nc.dram_tensor("wlin", [D, NL * D], f32, kind="ExternalInput")
    wint = nc.dram_tensor("wint", [D, NL * D], f32, kind="ExternalInput")
    biasc = nc.dram_tensor("biasc", [D, NL], f32, kind="ExternalInput")
    uidx = nc.dram_tensor("uidx", [128, NFB * 8], i16, kind="ExternalInput")
    iidx = nc.dram_tensor("iidx", [128, NFB * 8], i16, kind="ExternalInput")
    score = nc.dram_tensor("score", [128, NFB], f32, kind="ExternalOutput")
    fsh_out = nc.dram_tensor("fsh_out", [SHARD, D], bf16,
                             kind="ExternalOutput")

    add = mybir.AluOpType.add
    mult = mybir.AluOpType.mult
    is_equal = mybir.AluOpType.is_equal
    maxop = mybir.AluOpType.max
    AF = mybir.ActivationFunctionType

    with tile.TileContext(nc) as tc:
        with (
            tc.tile_pool(name="const", bufs=1) as cp,
            tc.tile_pool(name="ft", bufs=1) as ftp,
            tc.tile_pool(name="g", bufs=5) as gp,
            tc.tile_pool(name="fin", bufs=1) as fp_,
            tc.tile_pool(name="ot", bufs=6) as otp,
            tc.tile_pool(name="sb", bufs=3) as sbp,
            tc.tile_pool(name="psb", bufs=2, space="PSUM") as psb,
            tc.tile_pool(name="py", bufs=2, space="PSUM") as pyp,
            tc.tile_pool(name="pn", bufs=1, space="PSUM") as pnp,
            tc.tile_pool(name="pbc", bufs=1, space="PSUM") as pbc,
            tc.tile_pool(name="ptp", bufs=2, space="PSUM") as ptp,
            tc.tile_pool(name="dram", bufs=1, space="DRAM") as dp,
        ):
            # ---------- constants ----------
            iota_b = cp.tile([128, 2 * BLK], bf16)
            nc.gpsimd.iota(iota_b[:], pattern=[[1, 2 * BLK]], base=0,
                           channel_multiplier=0,
                           allow_small_or_imprecise_dtypes=True)
            ident = cp.tile([128, 128], bf16)
            make_identity(nc, ident[:])
            ones_f = cp.tile([128, 1], f32)
            nc.vector.memset(ones_f[:], 1.0)
            ones_b = cp.tile([128, 1], bf16)
            nc.vector.tensor_copy(ones_b[:], ones_f[:])
            onesrow_f = cp.tile([1, 128], f32)
            nc.vector.memset(onesrow_f[:], 1.0)
            onesrow_r = cp.tile([1, 128], f32r)
            nc.vector.tensor_copy(onesrow_r[:], onesrow_f[:])
            ones1_f = cp.tile([1, 1], f32)
            nc.vector.memset(ones1_f[:], 1.0)
            ones1_r = cp.tile([1, 1], f32r)
            nc.vector.tensor_copy(ones1_r[:], ones1_f[:])
            zrow_f = cp.tile([1, SB], f32)
            nc.vector.memset(zrow_f[:], 0.0)
            zrow_b = cp.tile([1, SB], f32r)
            nc.vector.tensor_copy(zrow_b[:], zrow_f[:])

            wlin_f = cp.tile([128, NL * 128], f32)
            nc.sync.dma_start(out=wlin_f[:], in_=wlin[:])
            wlin_r = cp.tile([128, NL * 128], bf16)
            nc.vector.tensor_copy(wlin_r[:], wlin_f[:])
            wint_f = cp.tile([128, NL * 128], f32)
            nc.sync.dma_start(out=wint_f[:], in_=wint[:])
            wint_r = cp.tile([128, NL * 128], bf16)
            nc.vector.tensor_copy(wint_r[:], wint_f[:])
            bias_sb = cp.tile([128, NL], f32)
            nc.sync.dma_start(out=bias_sb[:], in_=biasc[:])

            eidx_sb = cp.tile([128, NIC], i16)
            nc.sync.dma_start(out=eidx_sb[:], in_=eidx[:])
            erow_sb = cp.tile([128, NCH], f32)
            nc.sync.dma_start(out=erow_sb[:], in_=erow[:])
            evals_sb = cp.tile([128, NCH], f32)
            nc.sync.dma_start(out=evals_sb[:], in_=evals[:])
            uidx_sb = cp.tile([128, NFB * 8], i16)
            nc.sync.dma_start(out=uidx_sb[:], in_=uidx[:])
            iidx_sb = cp.tile([128, NFB * 8], i16)
            nc.sync.dma_start(out=iidx_sb[:], in_=iidx[:])

            fta = ftp.tile([128, NB * RB], bf16, tag="fta")
            ftb = ftp.tile([128, NB * RB], bf16, tag="ftb")
            nc.vector.memset(fta[:, SHARD:], 0.0)
            nc.sync.dma_start(out=fta[:, :SHARD], in_=f0t[:])
            lxt = ftp.tile([128, NB * RB], bf16, tag="lxt")
            nc.vector.memset(lxt[:, SHARD:], 0.0)

            fshard = dp.tile([SHARD, D], bf16)
            tabs = [
                dp.tile([N, D], bf16, name=f"tab{1 + i}", tag=f"tab{1 + i}",
                        addr_space="Shared")
                for i in range(NL)
            ]

            acc = cp.tile([128, NFB], f32)
            nc.vector.memset(acc[:], 0.0)

            n_layers = NLAYERS if STAGE >= 1 else 0
            for l in range(n_layers):
                ftin = fta if l % 2 == 0 else ftb
                ftout = ftb if l % 2 == 0 else fta
                src = tab0 if l == 0 else tabs[l - 1]

                # ---------- SpMM ----------
                gts = []
                for (rng, sb_i, c0, nq) in calls:
                    gt = gp.tile([128, nq * 128], bf16, tag="g",
                                 name=f"g{l}_{c0}")
                    nc.gpsimd.dma_gather(
                        gt[:].rearrange("p (c d) -> p c d", d=128),
                        src[rng * RS:, :],
                        eidx_sb[:, c0:c0 + nq * 8],
                        nq * 128, nq * 128, 128,
                        single_packet=False,
                    )
                    gts.append(gt)
                cur = None
                for qi, (call_i, sub_i, w, sb_open, sb_close, sb_i,
                         wide) in enumerate(chunks):
                    gt = gts[call_i]
                    if sb_open:
                        cur = psb.tile([128, SB], f32, tag="psb",
                                       name=f"psb{l}_{sb_i}")
                        nc.tensor.matmul(out=cur[:], lhsT=onesrow_r[:],
                                         rhs=zrow_b[:], start=True,
                                         stop=False)
                    ow = 2 * BLK if wide else BLK
                    o = otp.tile([128, 2 * BLK], bf16, tag="o",
                                 name=f"o{l}_{qi}")
                    nc.vector.tensor_scalar(
                        out=o[:, :ow], in0=iota_b[:, :ow],
                        scalar1=erow_sb[:, qi:qi + 1],
                        scalar2=evals_sb[:, qi:qi + 1],
                        op0=is_equal, op1=mult,
                    )
                    nc.tensor.matmul(
                        out=cur[:, w:w + ow],
                        lhsT=gt[:, ts(sub_i, 128)],
                        rhs=o[:, :ow],
                        start=False, stop=bool(sb_close),
                    )
                    if sb_close:
                        nsb = min(SB, SHARD - sb_i * SB)
                        nc.scalar.activation(
                            out=lxt[:, sb_i * SB:sb_i * SB + nsb],
                            in_=cur[:, :nsb], func=AF.Copy)

                # ---------- dense ----------
                for b in range(NB if STAGE >= 2 else 0):
                    lxs = lxt[:, ts(b, RB)]
                    fin_ = ftin[:, ts(b, RB)]
                    pre1 = sbp.tile([128, RB], bf16, tag="pre1")
                    nc.vector.tensor_tensor(out=pre1[:], in0=lxs, in1=fin_,
                                            op=add)
                    pre2 = sbp.tile([128, RB], bf16, tag="pre2")
                    nc.vector.tensor_tensor(out=pre2[:], in0=lxs, in1=fin_,
                                            op=mult)
                    y = pyp.tile([128, RB], f32, tag="y")
                    nc.tensor.matmul(out=y[:], lhsT=wlin_r[:, ts(l, 128)],
                                     rhs=pre1[:], start=True, stop=False)
                    nc.tensor.matmul(out=y[:], lhsT=wint_r[:, ts(l, 128)],
                                     rhs=pre2[:], start=False, stop=True)
                    ya = sbp.tile([128, RB], bf16, tag="ya")
                    nc.scalar.activation(out=ya[:], in_=y[:], func=AF.Lrelu,
                                         bias=bias_sb[:, l:l + 1], scale=1.0,
                                         alpha=SLOPE)
                    sq = sbp.tile([128, RB], bf16, tag="sq")
                    nc.vector.tensor_tensor(out=sq[:], in0=ya[:], in1=ya[:],
                                            op=mult)
                    nsq = pnp.tile([1, RB], f32, tag="nsq")
                    nc.tensor.matmul(out=nsq[:], lhsT=ones_b[:], rhs=sq[:],
                                     start=True, stop=True)
                    rt = sbp.tile([1, RB], f32, tag="rt")
                    nc.scalar.activation(out=rt[:], in_=nsq[:], func=AF.Sqrt)
                    nc.vector.tensor_scalar(out=rt[:], in0=rt[:],
                                            scalar1=EPS, scalar2=None,
                                            op0=maxop)
                    inv = sbp.tile([1, RB], f32r, tag="inv")
                    with nc.allow_low_precision(reason="f32r broadcast input"):
                        nc.vector.reciprocal(inv[:], rt[:])
                    bc = pbc.tile([128, RB], f32, tag="bc")
                    nc.tensor.matmul(out=bc[:], lhsT=onesrow_r[:], rhs=inv[:],
                                     start=True, stop=True)
                    nc.vector.tensor_tensor(out=ftout[:, ts(b, RB)],
                                            in0=ya[:], in1=bc[:], op=mult)
                    for h in range(4):
                        r0 = b * RB + h * 128
                        nrr = min(128, SHARD - r0)
                        if nrr <= 0:
                            break
                        tp = ptp.tile([128, 128], bf16, tag="tp",
                                      name=f"tp{l}_{b}_{h}")
                        nc.tensor.transpose(
                            out=tp[:], in_=ftout[:, r0:r0 + 128],
                            identity=ident[:])
                        cpo = sbp.tile([128, 128], bf16, tag="cpo")
                        nc.scalar.activation(out=cpo[:], in_=tp[:],
                                             func=AF.Copy)
                        nc.sync.dma_start(
                            out=(fshard if l < NL - 1 else fsh_out)
                            [r0:r0 + nrr, :],
                            in_=cpo[:nrr, :])

                if STAGE >= 3 and l < NL - 1:
                    nc.gpsimd.collective_compute(
                        "AllGather", mybir.AluOpType.bypass,
                        replica_groups=[list(range(NCORE))],
                        ins=[fshard.opt()], outs=[tabs[l].opt()],
                    )

            # ---------- finals: levels 1..NL-1, deferred past the last
            # layer's SpMM so their gathers fill DMA idle time instead of
            # competing right after each collective ----------
            for l in range(n_layers - 1 if STAGE >= 5 else 0):
                srcf = tabs[l]
                ug = fp_.tile([128, NFB * 128], bf16, tag="ug",
                              name=f"ug{l}")
                nc.gpsimd.dma_gather(
                    ug[:].rearrange("p (c d) -> p c d", d=128),
                    srcf[:],
                    uidx_sb[:],
                    NFB * 128, NFB * 128, 128,
                    single_packet=False,
                )
                ig = fp_.tile([128, NFB * 128], bf16, tag="ig",
                              name=f"ig{l}")
                for (rr, cc0, cn) in fin_bounds:
                    nc.gpsimd.dma_gather(
                        ig[:, cc0 * 128:(cc0 + cn) * 128].rearrange(
                            "p (c d) -> p c d", d=128),
                        srcf[rr * RS:, :],
                        iidx_sb[:, cc0 * 8:(cc0 + cn) * 8],
                        cn * 128, cn * 128, 128,
                        single_packet=False,
                    )
                nc.vector.tensor_tensor(out=ug[:], in0=ug[:],
                                        in1=ig[:], op=mult)
                sc = sbp.tile([128, NFB], f32, tag="sc")
                nc.vector.tensor_reduce(
                    out=sc[:],
                    in_=ug[:].rearrange("p (c d) -> p c d", d=128),
                    axis=mybir.AxisListType.X, op=add)
                nc.vector.tensor_tensor(out=acc[:], in0=acc[:],
                                        in1=sc[:], op=add)
            nc.sync.dma_start(out=score[:], in_=acc[:])

    nc.compile()
    return nc


def _pack_inputs(userIdx, itemIdx, rows, cols, vals, uEmbd, iEmbd,
                 Wlin, blin, Wint, bint):
    rows = np.asarray(rows, dtype=np.int64)
    cols = np.asarray(cols, dtype=np.int64)
    vals = np.asarray(vals, dtype=np.float32)
    userIdx = np.asarray(userIdx, dtype=np.int64)
    itemIdx = np.asarray(itemIdx, dtype=np.int64)

    feat0 = np.ascontiguousarray(
        np.concatenate([np.asarray(uEmbd, np.float32),
                        np.asarray(iEmbd, np.float32)], axis=0))
    tab0 = np.ascontiguousarray(feat0.astype(ml_dtypes.bfloat16))

    # ---- edge bucketing: cell = (core, blk, rng) ----
    core = rows // SHARD
    local = rows - core * SHARD
    blk = local // BLK
    rowl = (local - blk * BLK).astype(np.float32)
    rng = cols // RS
    col_local = (cols - rng * RS).astype(np.int16)

    ckey = ((core * NBLK + blk) * NR + rng).astype(np.int64)
    order = np.argsort(ckey, kind="stable")
    ckey_s = ckey[order]
    counts = np.bincount(ckey_s, minlength=NCORE * NBLK * NR)
    counts = counts.reshape(NCORE, NBLK, NR)

    # structure: gather call = (gsb, rng); psum group = sb of 512 rows.
    # Cells (blk128, rng) get floor(max8/128) full 128-wide chunks; the
    # leftovers of each aligned block PAIR (2b, 2b+1) share merged chunks
    # with a 256-wide one-hot window (256-aligned, so never straddling the
    # 512-col psum tile). Slot order within a call and chunk emission
    # order are identical: for sb -> for pair -> fulls(2b), fulls(2b+1),
    # merged.
    cnt_max = counts.max(axis=0)  # [NBLK, NR]
    F = cnt_max // 128            # full chunks per cell
    L = cnt_max - F * 128         # leftover per cell (0..127)
    NPAIR = NBLK // 2
    M = np.zeros((NPAIR, NR), dtype=np.int64)      # merged wide chunks
    SGL = np.zeros((NBLK, NR), dtype=np.int64)     # solo narrow leftover
    for p in range(NPAIR):
        a, b = 2 * p, 2 * p + 1
        do_merge = (L[a] > 0) & (L[b] > 0) & (L[a] + L[b] <= 128)
        M[p] = np.where(do_merge, 1, 0)
        SGL[a] = np.where(do_merge, 0, (L[a] > 0).astype(np.int64))
        SGL[b] = np.where(do_merge, 0, (L[b] > 0).astype(np.int64))

    FT = F + SGL        # narrow chunks per cell (fulls + solo leftover)
    calls = []          # (rng, gsb, idxcol0, n_slots)
    chunks = []         # (call_i, sub_i, w, sb_open, sb_close, sb_i, wide)
    full_sub = np.zeros((NBLK, NR), dtype=np.int64)   # sub_i of cell fulls
    merge_sub = np.zeros((NPAIR, NR), dtype=np.int64)  # sub_i of merged
    call_of_rng = {}
    idxcol = 0
    for g in range(NGSB):
        blk_g_lo = g * (GSB // BLK)
        blk_g_hi = min(NBLK, blk_g_lo + GSB // BLK)
        for r in range(NR):
            call_nq = 0
            for p in range(blk_g_lo // 2, (blk_g_hi + 1) // 2):
                for b in (2 * p, 2 * p + 1):
                    if b >= NBLK:
                        continue
                    full_sub[b, r] = call_nq
                    call_nq += int(FT[b, r])
                merge_sub[p, r] = call_nq
                call_nq += int(M[p, r])
            if call_nq == 0:
                continue
            call_of_rng[(g, r)] = (len(calls), idxcol)
            calls.append((r, g, idxcol, call_nq))
            idxcol += call_nq * 8
        # chunk emission: sb-major
        sb_lo = g * (GSB // SB)
        sb_hi = min(NSB, sb_lo + GSB // SB)
        for sbi in range(sb_lo, sb_hi):
            blk_lo = sbi * (SB // BLK)
            blk_hi = min(NBLK, blk_lo + SB // BLK)
            sb_chunks = []
            for r in range(NR):
                if (g, r) not in call_of_rng:
                    continue
                call_i, _ = call_of_rng[(g, r)]
                for p in range(blk_lo // 2, (blk_hi + 1) // 2):
                    for b in (2 * p, 2 * p + 1):
                        if b >= NBLK:
                            continue
                        for j in range(int(FT[b, r])):
                            sb_chunks.append(
                                (call_i, int(full_sub[b, r]) + j,
                                 (b - blk_lo) * BLK, False))
                    for j in range(int(M[p, r])):
                        sb_chunks.append(
                            (call_i, int(merge_sub[p, r]) + j,
                             (2 * p - blk_lo) * BLK, True))
            for t, (call_i, sub_i, w, wide) in enumerate(sb_chunks):
                chunks.append((call_i, sub_i, w, t == 0,
                               t == len(sb_chunks) - 1, sbi, wide))
    NCH = len(chunks)
    NIC = idxcol

    # ---- scatter edges into slots ----
    # per-cell chunk id (emission order) + idx col base for fulls; per-pair
    # for merged. A core's first 128*F[b,r] edges of cell (b,r) fill the
    # full chunks; remaining edges go to the pair's merged chunks after the
    # sibling cell's leftovers offset (per-core running offset).
    full_chunk = np.zeros((NBLK, NR), dtype=np.int64)
    full_icol = np.zeros((NBLK, NR), dtype=np.int64)
    merge_chunk = np.zeros((NPAIR, NR), dtype=np.int64)
    merge_icol = np.zeros((NPAIR, NR), dtype=np.int64)
    qi = 0
    for g in range(NGSB):
        sb_lo = g * (GSB // SB)
        sb_hi = min(NSB, sb_lo + GSB // SB)
        for sbi in range(sb_lo, sb_hi):
            blk_lo = sbi * (SB // BLK)
            blk_hi = min(NBLK, blk_lo + SB // BLK)
            for r in range(NR):
                if (g, r) not in call_of_rng:
                    continue
                call_i, ic0 = call_of_rng[(g, r)]
                for p in range(blk_lo // 2, (blk_hi + 1) // 2):
                    for b in (2 * p, 2 * p + 1):
                        if b >= NBLK:
                            continue
                        if FT[b, r]:
                            full_chunk[b, r] = qi
                            full_icol[b, r] = ic0 + int(full_sub[b, r]) * 8
                        qi += int(FT[b, r])
                    if M[p, r]:
                        merge_chunk[p, r] = qi
                        merge_icol[p, r] = ic0 + int(merge_sub[p, r]) * 8
                    qi += int(M[p, r])
    assert qi == NCH, (qi, NCH)

    starts = np.zeros(NCORE * NBLK * NR, dtype=np.int64)
    np.cumsum(counts.reshape(-1)[:-1], out=starts[1:])
    pos = np.arange(len(ckey_s), dtype=np.int64) - starts[ckey_s]
    core_s = core[order]
    blk_s = blk[order]
    rng_s = rng[order]
    pair_s = blk_s // 2
    fcap = (FT * 128)[blk_s, rng_s]         # narrow-chunk capacity of cell
    is_full = pos < fcap
    # merged position: sibling-leftover offset for odd blocks (per core)
    sib = blk_s ^ 1
    sib_left = np.where(
        (blk_s % 2 == 1) & (sib < NBLK),
        np.maximum(counts[core_s, sib, rng_s] - (FT * 128)[sib, rng_s], 0),
        0)
    mpos = np.where(is_full, 0, pos - fcap + sib_left)
    q_of_edge = np.where(
        is_full,
        full_chunk[blk_s, rng_s] + pos // 128,
        merge_chunk[pair_s, rng_s] + mpos // 128)
    icol_of_edge = np.where(
        is_full,
        full_icol[blk_s, rng_s] + (pos // 16) % 8 + (pos // 128) * 8,
        merge_icol[pair_s, rng_s] + (mpos // 16) % 8 + (mpos // 128) * 8)
    p_of_edge = np.where(is_full, pos % 128, mpos % 128)
    # erow: window-relative dest. fulls: blk-local [0,128); merged: relative
    # to the pair's even block [0,256)
    roww = np.where(is_full, rowl[order],
                    rowl[order] + (blk_s % 2) * 128)

    eidx_arr = np.zeros((NCORE, 16, NIC), dtype=np.int16)
    erow_arr = np.zeros((NCORE, 128, NCH), dtype=np.float32)
    eval_arr = np.zeros((NCORE, 128, NCH), dtype=np.float32)
    eidx_arr[core_s, p_of_edge % 16, icol_of_edge] = col_local[order]
    erow_arr[core_s, p_of_edge, q_of_edge] = roww
    eval_arr[core_s, p_of_edge, q_of_edge] = vals[order]

    # ---- weights ----
    wlin_h = np.ascontiguousarray(
        np.asarray(Wlin, np.float32).transpose(1, 0, 2).reshape(D, NL * D))
    wint_h = np.ascontiguousarray(
        np.asarray(Wint, np.float32).transpose(1, 0, 2).reshape(D, NL * D))
    biasc = np.ascontiguousarray(
        (np.asarray(blin, np.float32) + np.asarray(bint, np.float32)).T)

    # ---- final stage: bucket item rows by range ----
    irow = itemIdx + NUM_USERS
    ir = irow // RS
    nfb_counts = np.zeros((NCORE, NR), dtype=np.int64)
    perms = []
    for c in range(NCORE):
        sl = slice(c * BSH, (c + 1) * BSH)
        o = np.argsort(ir[sl], kind="stable")
        perms.append(o)
        nfb_counts[c] = np.bincount(ir[sl][o], minlength=NR)
    bucket_chunks = np.ceil(nfb_counts.max(axis=0) / 128).astype(np.int64)
    fin_bounds = []
    c0 = 0
    for r in range(NR):
        n = int(bucket_chunks[r])
        if n == 0:
            continue
        fin_bounds.append((r, c0, n))
        c0 += n
    NFB = c0

    uidx_arr = np.zeros((NCORE, 16, NFB * 8), dtype=np.int16)
    iidx_arr = np.zeros((NCORE, 16, NFB * 8), dtype=np.int16)
    inv_perm = np.full((NCORE, NFB * 128), -1, dtype=np.int64)
    for c in range(NCORE):
        sl = slice(c * BSH, (c + 1) * BSH)
        o = perms[c]
        u_s = userIdx[sl][o]
        i_s = irow[sl][o]
        r_s = ir[sl][o]
        jpos = np.zeros(BSH, dtype=np.int64)
        for (r, b0, nchk) in fin_bounds:
            m = r_s == r
            jpos[m] = b0 * 128 + np.arange(int(m.sum()))
        uidx_arr[c, jpos % 16, (jpos // 128) * 8 + (jpos % 128) // 16] = \
            u_s.astype(np.int16)
        iidx_arr[c, jpos % 16, (jpos // 128) * 8 + (jpos % 128) // 16] = \
            (i_s - r_s * RS).astype(np.int16)
        inv_perm[c, jpos] = np.arange(c * BSH, (c + 1) * BSH)[o]

    # host-side level-0 contribution (exact f32); level-3 added after run
    score0 = np.sum(feat0[userIdx] * feat0[irow], axis=1).astype(np.float32)

    meta = (tuple(calls), tuple(chunks), NIC, NCH, NFB, tuple(fin_bounds))

    in_maps = []
    for c in range(NCORE):
        f0t = np.ascontiguousarray(
            feat0[c * SHARD:(c + 1) * SHARD].T.astype(ml_dtypes.bfloat16))
        in_maps.append({
            "tab0": tab0,
            "f0t": f0t,
            "eidx": np.ascontiguousarray(np.tile(eidx_arr[c], (8, 1))),
            "erow": np.ascontiguousarray(erow_arr[c]),
            "evals": np.ascontiguousarray(eval_arr[c]),
            "wlin": wlin_h,
            "wint": wint_h,
            "biasc": biasc,
            "uidx": np.ascontiguousarray(np.tile(uidx_arr[c], (8, 1))),
            "iidx": np.ascontiguousarray(np.tile(iidx_arr[c], (8, 1))),
        })
    return meta, in_maps, inv_perm, score0


def kernel(**inputs) -> np.ndarray:
    meta, in_maps, inv_perm, score0 = _pack_inputs(**inputs)
    key = meta[:4] + (meta[4], meta[5])
    if key not in _cache:
        _cache[key] = _build(meta)
    nc = _cache[key]
    res = run_bass_kernel_spmd(nc, in_maps, list(range(NCORE)))
    out = np.empty(BATCH, dtype=np.float32)
    NFB = meta[4]
    for c in range(NCORE):
        sc = res.results[c]["score"]  # [128, NFB]
        vals_j = sc[np.arange(NFB * 128) % 128, np.arange(NFB * 128) // 128]
        valid = inv_perm[c] >= 0
        out[inv_perm[c][valid]] = vals_j[valid]
    f3 = np.concatenate(
        [np.asarray(res.results[c]["fsh_out"]).astype(np.float32)
         for c in range(NCORE)], axis=0)
    userIdx = np.asarray(inputs["userIdx"], dtype=np.int64)
    itemIdx = np.asarray(inputs["itemIdx"], dtype=np.int64)
    score3 = np.sum(f3[userIdx] * f3[itemIdx + NUM_USERS], axis=1)
    return out + score0 + score3.astype(np.float32)
